# revision 1
# baseline (speedup 1.0000x reference)
"""Trainium2 Bass kernel: 12-layer BERT-base forward + per-sample annotator
head. Data-parallel across 8 NeuronCores (8 sequences / 2048 tokens per core,
no collectives).

Layout: feature-major activations [128 partitions, 6 chunks, T tokens].
bf16 matmuls with f32 PSUM accumulation. The f32 residual master lives in
DRAM and streams through SBUF per 512-token block. LayerNorm stats and
softmax denominators use ones-matrix matmuls (producing partition-replicated
rows at full DVE lane utilization). Attention is computed as scoresT [k, q]
(keys on partitions) so softmax/ctx need no transposes anywhere.

Self-contained: the harness calls kernel(**inputs) with the full unsharded
inputs; sharding/gather happens on the host here.
"""
import os
import numpy as np
import ml_dtypes

import concourse.bass as bass
import concourse.mybir as mybir
from concourse.tile import TileContext
from concourse.bass_utils import run_bass_kernel_spmd

# model dims (hardcoded per problem spec)
B, S, H, NLAYER, NH, VOC, ANN, NL = 64, 256, 768, 12, 12, 30522, 64, 2
HD = H // NH            # 64
FF = 4 * H              # 3072
P = 128
CH = H // P             # 6
FCH = FF // P           # 24
NCORES = 8
NB = B // NCORES        # 8 sequences per core
T = NB * S              # 2048 tokens per core
TB = 512                # token block
NTB = T // TB           # 4
PAIR = 2 * S            # 512 tokens per sequence-pair
EPS = 1e-12

F32 = mybir.dt.float32
BF16 = mybir.dt.bfloat16
AF = mybir.ActivationFunctionType
ALU = mybir.AluOpType

_NLAYERS = int(os.environ.get("KERNEL_LAYERS", str(NLAYER)))


# ---------------------------------------------------------------- wait split
def _split_sync_waits(nc, max_waits=1):
    """This walrus build rejects >~2 sync waits on one instruction; move
    overflow waits onto wait-only NoOps inserted before, same engine."""
    ctr = 0
    for f in nc.m.functions:
        for bb in f.blocks:
            new_list, changed = [], False
            for inst in bb.instructions:
                si = inst.sync_info
                waits = list(si.on_wait) if si and si.on_wait else []
                if len(waits) > max_waits:
                    changed = True
                    overflow = waits[: len(waits) - max_waits]
                    keep = waits[len(waits) - max_waits:]
                    for i in range(0, len(overflow), max_waits):
                        ctr += 1
                        nop = mybir.InstNoOp(name=f"waitsplit-{ctr}")
                        nop.engine = inst.engine
                        nop.sync_info = mybir.SyncInfo(
                            on_wait=overflow[i:i + max_waits], on_update=[])
                        nc.register_instruction(nop)
                        new_list.append(nop)
                    si.on_wait = keep
                    inst.sync_info = si
                new_list.append(inst)
            if changed:
                bb.instructions = new_list


# ---------------------------------------------------------------- host prep
def _tile_kxo(w, k, o):
    # [k, o] -> [128, k/128, o/128, 128] (kp, ko, oo, oc)
    return np.ascontiguousarray(
        w.reshape(k // P, P, o // P, P).transpose(1, 0, 2, 3))


def _rows_k(w, k, o):
    # [k, o] -> [128, k/128, o] (kp, ko, o)
    return np.ascontiguousarray(w.reshape(k // P, P, o).transpose(1, 0, 2))


def _w2_grouped(w):
    # [FF, H] -> [H/128 (oo), 128 (kp), FF/128 (ko), 128 (oc)]
    return np.ascontiguousarray(
        w.reshape(FCH, P, CH, P).transpose(2, 1, 0, 3))


def _bf(x):
    return np.asarray(x, np.float32).astype(ml_dtypes.bfloat16)


# ---------------------------------------------------------------- builder
def build(nl: int):
    nc = bass.Bass(target_bir_lowering=False)

    h0_d = nc.declare_dram_parameter("h0", [P, CH, T], F32, isOutput=False)
    wq_d = nc.declare_dram_parameter("wq", [nl, P, CH, CH, P], BF16, isOutput=False)
    wk_d = nc.declare_dram_parameter("wk", [nl, P, CH, CH, P], BF16, isOutput=False)
    wv_d = nc.declare_dram_parameter("wv", [nl, P, CH, H], BF16, isOutput=False)
    wo_d = nc.declare_dram_parameter("wo", [nl, P, CH, CH, P], BF16, isOutput=False)
    w1_d = nc.declare_dram_parameter("w1", [nl, P, CH, FCH, P], BF16, isOutput=False)
    w2_d = nc.declare_dram_parameter("w2", [nl, CH, P, FCH, P], BF16, isOutput=False)
    hw_d = nc.declare_dram_parameter("hw", [P, CH, 2 * NB], F32, isOutput=False)
    out_d = nc.declare_dram_parameter("out", [NB, 2 * NB], F32, isOutput=True)

    from contextlib import ExitStack
    with TileContext(nc) as tc:
        with ExitStack() as ctx:
            persist = ctx.enter_context(tc.tile_pool(name="persist", bufs=1))
            dram_pool = ctx.enter_context(tc.tile_pool(name="dram", bufs=1, space="DRAM"))
            big_pool = ctx.enter_context(tc.tile_pool(name="big", bufs=2))
            qkv_pool = ctx.enter_context(tc.tile_pool(name="qkvp", bufs=1))
            w768_pool = ctx.enter_context(tc.tile_pool(name="w768", bufs=3))
            w1_pool = ctx.enter_context(tc.tile_pool(name="w1s", bufs=2))
            w2_pool = ctx.enter_context(tc.tile_pool(name="w2s", bufs=2))
            g_pool = ctx.enter_context(tc.tile_pool(name="gp", bufs=1))
            blk_pool = ctx.enter_context(tc.tile_pool(name="blk", bufs=3))
            lnbf_pool = ctx.enter_context(tc.tile_pool(name="lnbf", bufs=2))
            lnst_pool = ctx.enter_context(tc.tile_pool(name="lnst", bufs=1))
            rec_pool = ctx.enter_context(tc.tile_pool(name="recp", bufs=2))
            small_pool = ctx.enter_context(tc.tile_pool(name="small", bufs=6))
            attn_pool = ctx.enter_context(tc.tile_pool(name="attn", bufs=3))
            ps_mm = ctx.enter_context(tc.tile_pool(name="psmm", bufs=3, space="PSUM"))
            ps_attn = ctx.enter_context(tc.tile_pool(name="psattn", bufs=5, space="PSUM"))

            ones_b = persist.tile([P, P], BF16)
            nc.vector.memset(ones_b[:], 1.0)
            eps_t = persist.tile([P, 1], F32)
            nc.vector.memset(eps_t[:], EPS)
            mst = dram_pool.tile([P, CH, T], F32)

            def ln_stats(src):
                """Per-token mean/rstd of src [P,CH,TB] f32, replicated on
                all partitions via ones-matrix matmuls. Returns (mneg, a_t)."""
                HB = TB // 2
                ps_s = ps_mm.tile([P, TB], F32, tag="pm")
                ps_ss = ps_mm.tile([P, TB], F32, tag="pm")
                for hf in range(2):
                    hsl = slice(hf * HB, (hf + 1) * HB)
                    srcb = lnst_pool.tile([P, CH, HB], BF16, tag="srcb")
                    nc.vector.tensor_copy(srcb[:], src[:, :, hsl])
                    sqb = lnst_pool.tile([P, CH, HB], BF16, tag="sqb")
                    nc.vector.tensor_tensor(sqb[:], src[:, :, hsl],
                                            src[:, :, hsl], ALU.mult)
                    for c in range(CH):
                        nc.tensor.matmul(ps_s[:, hsl], ones_b[:], srcb[:, c],
                                         start=(c == 0), stop=(c == CH - 1))
                    for c in range(CH):
                        nc.tensor.matmul(ps_ss[:, hsl], ones_b[:], sqb[:, c],
                                         start=(c == 0), stop=(c == CH - 1))
                mneg = small_pool.tile([P, TB], F32, tag="st")
                nc.vector.tensor_scalar_mul(mneg[:], ps_s[:], -1.0 / H)
                msq = small_pool.tile([P, TB], F32, tag="st")
                nc.vector.tensor_tensor(msq[:], mneg[:], mneg[:], ALU.mult)
                var = small_pool.tile([P, TB], F32, tag="st")
                nc.vector.scalar_tensor_tensor(var[:], ps_ss[:], 1.0 / H,
                                               msq[:], ALU.mult, ALU.subtract)
                lnv = small_pool.tile([P, TB], F32, tag="st")
                nc.scalar.activation(lnv[:], var[:], AF.Ln,
                                     bias=eps_t[:, 0:1])
                a_t = small_pool.tile([P, TB], F32, tag="st")
                nc.scalar.activation(a_t[:], lnv[:], AF.Exp, scale=-0.5)
                return mneg, a_t

            def ln_apply(src, mneg, a_t, out_f32=None, out_bf=None):
                """(src - mean) * rstd -> out_f32 and/or out_bf. Per-chunk 2D
                ops (3D broadcast APs drop DVE to 1x mode)."""
                tmp = blk_pool.tile([P, CH, TB], F32, tag="blk")
                for c in range(CH):
                    nc.vector.tensor_tensor(tmp[:, c], src[:, c], mneg[:],
                                            ALU.add)
                for c in range(CH):
                    if out_f32 is not None:
                        nc.vector.tensor_tensor(out_f32[:, c], tmp[:, c],
                                                a_t[:], ALU.mult)
                    else:
                        nc.vector.tensor_tensor(out_bf[:, c], tmp[:, c],
                                                a_t[:], ALU.mult)
                if out_f32 is not None and out_bf is not None:
                    nc.scalar.activation(out_bf, out_f32[:], AF.Copy)

            def layernorm(src, out_f32=None, out_bf=None):
                mneg, a_t = ln_stats(src)
                ln_apply(src, mneg, a_t, out_f32, out_bf)

            h_bf = big_pool.tile([P, CH, T], BF16, tag="big")

            for l in range(nl):
                wq_t = w768_pool.tile([P, CH, CH, P], BF16, tag="w768")
                nc.sync.dma_start(wq_t[:], wq_d[l])
                wk_t = w768_pool.tile([P, CH, CH, P], BF16, tag="w768")
                nc.sync.dma_start(wk_t[:], wk_d[l])
                wv_t = w768_pool.tile([P, CH, H], BF16, tag="w768")
                nc.sync.dma_start(wv_t[:], wv_d[l])
                ctx_sb = big_pool.tile([P, CH, T], BF16, tag="big")

                if l == 0:
                    for tb0 in range(NTB):
                        psl = slice(tb0 * TB, (tb0 + 1) * TB)
                        src0 = blk_pool.tile([P, CH, TB], F32, tag="blk")
                        nc.sync.dma_start(src0[:], h0_d[:, :, psl])
                        ln0f = blk_pool.tile([P, CH, TB], F32, tag="blk")
                        layernorm(src0, out_f32=ln0f, out_bf=h_bf[:, :, psl])
                        nc.sync.dma_start(mst[:, :, psl], ln0f[:])

                def do_pair(pr, l=l, h_bf=h_bf, ctx_sb=ctx_sb,
                            wq_t=wq_t, wk_t=wk_t, wv_t=wv_t):
                    psl = slice(pr * PAIR, (pr + 1) * PAIR)
                    qt_b = qkv_pool.tile([P, CH, PAIR], BF16, tag="qtb")
                    kt_b = qkv_pool.tile([P, CH, PAIR], BF16, tag="ktb")
                    v_b = qkv_pool.tile([P, 2, 2, NH, HD], BF16, tag="vb")
                    for w_t, dst in ((wq_t, qt_b), (wk_t, kt_b)):
                        for o in range(CH):
                            ps = ps_mm.tile([P, TB], F32, tag="pm")
                            for k in range(CH):
                                nc.tensor.matmul(ps[:], w_t[:, k, o],
                                                 h_bf[:, k, psl],
                                                 start=(k == 0),
                                                 stop=(k == CH - 1))
                            nc.vector.tensor_copy(dst[:, o], ps[:])
                    for ci in range(PAIR // P):
                        csl = slice(pr * PAIR + ci * P,
                                    pr * PAIR + (ci + 1) * P)
                        bi, kt_i = ci // 2, ci % 2
                        for dh in range(2):
                            ps = ps_mm.tile([P, TB], F32, tag="pm")
                            for k in range(CH):
                                nc.tensor.matmul(
                                    ps[:, : H // 2],
                                    h_bf[:, k, csl],
                                    wv_t[:, k, dh * (H // 2):(dh + 1) * (H // 2)],
                                    start=(k == 0), stop=(k == CH - 1))
                            nc.vector.tensor_copy(
                                v_b[:, bi, kt_i, dh * 6:(dh + 1) * 6],
                                ps[:, : H // 2].rearrange(
                                    "p (h d) -> p h d", d=HD))
                    # attention: software-pipelined over the 24 (bi, h) heads
                    heads = [(bi, h) for bi in range(2) for h in range(NH)]
                    pend = {}
                    for i in range(len(heads) + 1):
                        if i < len(heads):
                            bi, h = heads[i]
                            po = (h % 2) * HD
                            c = h // 2
                            qsl = slice(bi * S, (bi + 1) * S)
                            at = attn_pool.tile([P, 2, S], BF16, tag="at")
                            for kt_i in range(2):
                                ksl = slice(bi * S + kt_i * P,
                                            bi * S + (kt_i + 1) * P)
                                ps_sc = ps_attn.tile([P, S], F32, tag="pa")
                                nc.tensor.matmul(
                                    ps_sc[:], kt_b[po:po + HD, c, ksl],
                                    qt_b[po:po + HD, c, qsl],
                                    start=True, stop=True,
                                    tile_position=(po, 0))
                                nc.scalar.activation(at[:, kt_i], ps_sc[:],
                                                     AF.Exp,
                                                     scale=1.0 / np.sqrt(HD))
                            pend[i] = at
                        if i >= 1:
                            bi, h = heads[i - 1]
                            at = pend.pop(i - 1)
                            po = (h % 2) * HD
                            c = h // 2
                            gsl = slice(pr * PAIR + bi * S,
                                        pr * PAIR + (bi + 1) * S)
                            ps_sum = ps_attn.tile([P, S], F32, tag="pa")
                            for kt_i in range(2):
                                nc.tensor.matmul(ps_sum[:], ones_b[:],
                                                 at[:, kt_i],
                                                 start=(kt_i == 0),
                                                 stop=(kt_i == 1))
                            lns = rec_pool.tile([P, S], F32, tag="rec")
                            nc.scalar.activation(lns[:], ps_sum[:], AF.Ln)
                            rec = rec_pool.tile([P, S], F32, tag="rec")
                            nc.scalar.activation(rec[:], lns[:], AF.Exp,
                                                 scale=-1.0)
                            ps_ctx = ps_attn.tile([P, S], F32, tag="pa")
                            for kt_i in range(2):
                                nc.tensor.matmul(ps_ctx[po:po + HD],
                                                 v_b[:, bi, kt_i, h],
                                                 at[:, kt_i],
                                                 start=(kt_i == 0),
                                                 stop=(kt_i == 1),
                                                 tile_position=(0, po))
                            nc.vector.tensor_tensor(
                                ctx_sb[po:po + HD, c, gsl],
                                ps_ctx[po:po + HD], rec[po:po + HD],
                                ALU.mult)

                wo_t = w768_pool.tile([P, CH, CH, P], BF16, tag="w768")
                nc.sync.dma_start(wo_t[:], wo_d[l])
                hbf_next = big_pool.tile([P, CH, T], BF16, tag="big")

                def residual1(tb, ctx_sb=ctx_sb, wo_t=wo_t):
                    sl = slice(tb * TB, (tb + 1) * TB)
                    m_in = blk_pool.tile([P, CH, TB], F32, tag="blk")
                    nc.sync.dma_start(m_in[:], mst[:, :, sl])
                    hpre = blk_pool.tile([P, CH, TB], F32, tag="blk")
                    for o in range(CH):
                        ps = ps_mm.tile([P, TB], F32, tag="pm")
                        for k in range(CH):
                            nc.tensor.matmul(ps[:], wo_t[:, k, o],
                                             ctx_sb[:, k, sl],
                                             start=(k == 0), stop=(k == CH - 1))
                        nc.vector.scalar_tensor_tensor(
                            hpre[:, o], ps[:], 1.0, m_in[:, o],
                            ALU.mult, ALU.add)
                    return hpre

                cd_state = {}

                def do_cd(tb, l=l, hbf_next=hbf_next, residual1=residual1,
                          cd_state=cd_state):
                    sl = slice(tb * TB, (tb + 1) * TB)
                    if tb == 0:
                        cd_state["hpre"] = residual1(0)
                    hpre = cd_state["hpre"]
                    mneg1, a1 = ln_stats(hpre)
                    ln1_bf = lnbf_pool.tile([P, CH, TB], BF16, tag="lnbf")
                    ln_apply(hpre, mneg1, a1, out_bf=ln1_bf[:])
                    if tb + 1 < NTB:
                        cd_state["hpre"] = residual1(tb + 1)
                    g = g_pool.tile([P, FCH, TB], BF16, tag="g")
                    for fog in range(FCH // 2):
                        w1_t = w1_pool.tile([P, CH, 2, P], BF16, tag="w1")
                        nc.sync.dma_start(
                            w1_t[:], w1_d[l, :, :, fog * 2:(fog + 1) * 2, :])
                        for fi in range(2):
                            fo = fog * 2 + fi
                            ps = ps_mm.tile([P, TB], F32, tag="pm")
                            for k in range(CH):
                                nc.tensor.matmul(ps[:], w1_t[:, k, fi],
                                                 ln1_bf[:, k],
                                                 start=(k == 0),
                                                 stop=(k == CH - 1))
                            nc.scalar.activation(g[:, fo], ps[:], AF.Gelu)
                    hpre2 = blk_pool.tile([P, CH, TB], F32, tag="blk")
                    for o in range(CH):
                        w2_t = w2_pool.tile([P, FCH, P], BF16, tag="w2")
                        nc.sync.dma_start(w2_t[:], w2_d[l, o])
                        ps = ps_mm.tile([P, TB], F32, tag="pm")
                        for k in range(FCH):
                            nc.tensor.matmul(ps[:], w2_t[:, k], g[:, k],
                                             start=(k == 0),
                                             stop=(k == FCH - 1))
                        nc.vector.scalar_tensor_tensor(
                            hpre2[:, o], ps[:], 1.0, ln1_bf[:, o],
                            ALU.mult, ALU.add)
                    mneg2, a2 = ln_stats(hpre2)
                    ln2f = blk_pool.tile([P, CH, TB], F32, tag="blk")
                    ln_apply(hpre2, mneg2, a2, out_f32=ln2f,
                             out_bf=hbf_next[:, :, sl])
                    nc.sync.dma_start(mst[:, :, sl], ln2f[:])

                for pr in range(T // PAIR):
                    do_pair(pr)
                for tb in range(NTB):
                    do_cd(tb)
                h_bf = hbf_next

            # ---- head ----
            hw_sb = persist.tile([P, CH, 2 * NB], F32)
            nc.sync.dma_start(hw_sb[:], hw_d[:])
            cls = persist.tile([P, CH, NB], F32)
            nc.sync.dma_start(cls[:], mst[:, :, 0:T:S])
            ps = ps_attn.tile([P, 2 * NB], F32, tag="pa")
            for c in range(CH):
                nc.tensor.matmul(ps[0:NB], cls[:, c], hw_sb[:, c],
                                 start=(c == 0), stop=(c == CH - 1))
            res = persist.tile([NB, 2 * NB], F32)
            nc.scalar.activation(res[:], ps[0:NB], AF.Copy)
            nc.sync.dma_start(out_d[:], res[:])

    _split_sync_waits(nc, max_waits=1)
    return nc


def _prep_weights(inputs, nl):
    wq = np.stack([_tile_kxo(_bf(inputs["Wq"][i]), H, H) for i in range(nl)])
    wk = np.stack([_tile_kxo(_bf(inputs["Wk"][i]), H, H) for i in range(nl)])
    wv = np.stack([_rows_k(_bf(inputs["Wv"][i]), H, H) for i in range(nl)])
    wo = np.stack([_tile_kxo(_bf(inputs["Wo"][i]), H, H) for i in range(nl)])
    w1 = np.stack([_tile_kxo(_bf(inputs["W1"][i]), H, FF) for i in range(nl)])
    w2 = np.stack([_w2_grouped(_bf(inputs["W2"][i])) for i in range(nl)])
    return wq, wk, wv, wo, w1, w2


def kernel(**inputs):
    nl = _NLAYERS
    for name in ("bq", "bk", "bv", "bo", "b1", "b2", "emb_ln_b", "head_b",
                 "ln1_b", "ln2_b"):
        assert not np.any(np.asarray(inputs[name])), f"{name} nonzero: unsupported"
    for name in ("emb_ln_s", "ln1_s", "ln2_s"):
        assert np.all(np.asarray(inputs[name]) == 1.0), f"{name}!=1: unsupported"
    assert np.all(np.asarray(inputs["attention_mask"]) == 1), "mask unsupported"

    ids = np.asarray(inputs["input_ids"])
    tt = np.asarray(inputs["token_type_ids"])
    we = np.asarray(inputs["word_emb"], np.float32)
    pe = np.asarray(inputs["pos_emb"], np.float32)
    te = np.asarray(inputs["type_emb"], np.float32)
    annot = np.asarray(inputs["annotator_idx"])
    hW = np.asarray(inputs["head_W"], np.float32)

    emb = we[ids] + pe[:S][None] + te[tt]          # [B, S, H] f32
    wq, wk, wv, wo, w1, w2 = _prep_weights(inputs, nl)

    in_maps = []
    for c in range(NCORES):
        e = emb[c * NB:(c + 1) * NB].reshape(T, CH, P).transpose(2, 1, 0)
        hw_g = hW[annot[c * NB:(c + 1) * NB]]       # [NB, H, 2]
        hwt = hw_g.transpose(1, 0, 2).reshape(H, 2 * NB) \
            .reshape(CH, P, 2 * NB).transpose(1, 0, 2)
        in_maps.append({
            "h0": np.ascontiguousarray(e),
            "wq": wq, "wk": wk, "wv": wv, "wo": wo, "w1": w1, "w2": w2,
            "hw": np.ascontiguousarray(hwt),
        })

    nc = build(nl)

    trace = bool(int(os.environ.get("KERNEL_TRACE", "0")))
    kwargs = {}
    if trace:
        try:
            import profshim
            profshim.install()
            kwargs["tmpdir"] = os.environ.get("KERNEL_TRACE_DIR")
        except Exception:
            trace = False
    res = run_bass_kernel_spmd(nc, in_maps, core_ids=list(range(NCORES)),
                               trace=trace, **kwargs)
    kernel.last_exec_time_ns = res.exec_time_ns

    out = np.zeros((B, NL), np.float32)
    for c in range(NCORES):
        oc = res.results[c]["out"]                 # [NB, 2*NB]
        for b in range(NB):
            out[c * NB + b] = oc[b, 2 * b:2 * b + 2]
    return out



# revision 6
# speedup vs baseline: 1.0544x; 1.0544x over previous
"""Trainium2 Bass kernel v2: 12-layer BERT-base forward + per-sample annotator
head. Data-parallel across 8 NeuronCores (8 sequences / 2048 tokens per core,
no collectives).

v2 changes vs baseline:
- fp16 end-to-end (weights, activations, residual master). fp16's 10-bit
  mantissa keeps rel err ~2e-3 (validated in numpy) while enabling 2x packed
  DVE modes and a 16-bit residual master.
- The LN2 output `xs` is BOTH the GEMM input and the residual master, kept
  resident in SBUF and updated in place per 512-token block: the f32 DRAM
  residual stream of the baseline is gone entirely.
- LayerNorm = center (fp16 2x) + square (fp16 2x) + scale (fp16 2x) with
  sum/sumsq via ones-matmuls; rstd via Ln/Exp on scalar engine.
- Attention processed in head-PAIRS so the K=64 score matmuls alternate PE
  row-groups (0/64) and the M=64 ctx matmuls alternate col-groups, which the
  PE runs concurrently. exp over a merged [P,2S] tile; softmax reciprocal
  via Ln/Exp (stays in the natural_log_exp table set, no table thrash).
- Engine balance: QKV/ctx evacuations on DVE, exp/ln/gelu on scalar,
  residual adds as single fused scalar_tensor_tensor ops.
"""
import os
import numpy as np

import concourse.bass as bass
import concourse.mybir as mybir
from concourse.tile import TileContext
from concourse.bass_utils import run_bass_kernel_spmd

# model dims (hardcoded per problem spec)
B, S, H, NLAYER, NH, VOC, ANN, NL = 64, 256, 768, 12, 12, 30522, 64, 2
HD = H // NH            # 64
FF = 4 * H              # 3072
P = 128
CH = H // P             # 6
FCH = FF // P           # 24
NCORES = 8
NB = B // NCORES        # 8 sequences per core
T = NB * S              # 2048 tokens per core
TB = 512                # token block
NTB = T // TB           # 4
PAIR = 2 * S            # 512 tokens per sequence-pair (== TB)
EPS = 1e-12

F32 = mybir.dt.float32
F16 = mybir.dt.float16
AF = mybir.ActivationFunctionType
ALU = mybir.AluOpType

_NLAYERS = int(os.environ.get("KERNEL_LAYERS", str(NLAYER)))


# ---------------------------------------------------------------- wait split
def _split_sync_waits(nc, max_waits=1):
    """This walrus build rejects >~2 sync waits on one instruction; move
    overflow waits onto wait-only NoOps inserted before, same engine."""
    ctr = 0
    for f in nc.m.functions:
        for bb in f.blocks:
            new_list, changed = [], False
            for inst in bb.instructions:
                si = inst.sync_info
                waits = list(si.on_wait) if si and si.on_wait else []
                if len(waits) > max_waits:
                    changed = True
                    overflow = waits[: len(waits) - max_waits]
                    keep = waits[len(waits) - max_waits:]
                    for i in range(0, len(overflow), max_waits):
                        ctr += 1
                        nop = mybir.InstNoOp(name=f"waitsplit-{ctr}")
                        nop.engine = inst.engine
                        nop.sync_info = mybir.SyncInfo(
                            on_wait=overflow[i:i + max_waits], on_update=[])
                        nc.register_instruction(nop)
                        new_list.append(nop)
                    si.on_wait = keep
                    inst.sync_info = si
                new_list.append(inst)
            if changed:
                bb.instructions = new_list


# ---------------------------------------------------------------- host prep
def _tile_kxo(w, k, o):
    # [k, o] -> [128, k/128, o/128, 128] (kp, ko, oo, oc)
    return np.ascontiguousarray(
        w.reshape(k // P, P, o // P, P).transpose(1, 0, 2, 3))


def _rows_k(w, k, o):
    # [k, o] -> [128, k/128, o] (kp, ko, o)
    return np.ascontiguousarray(w.reshape(k // P, P, o).transpose(1, 0, 2))


def _w2_grouped(w):
    # [FF, H] -> [H/128 (oo), 128 (kp), FF/128 (ko), 128 (oc)]
    return np.ascontiguousarray(
        w.reshape(FCH, P, CH, P).transpose(2, 1, 0, 3))


def _f16(x):
    return np.asarray(x, np.float32).astype(np.float16)


# ---------------------------------------------------------------- builder
def build(nl: int):
    nc = bass.Bass(target_bir_lowering=False)

    h0_d = nc.declare_dram_parameter("h0", [P, CH, T], F16, isOutput=False)
    wq_d = nc.declare_dram_parameter("wq", [nl, P, CH, CH, P], F16, isOutput=False)
    wk_d = nc.declare_dram_parameter("wk", [nl, P, CH, CH, P], F16, isOutput=False)
    wv_d = nc.declare_dram_parameter("wv", [nl, P, CH, H], F16, isOutput=False)
    wo_d = nc.declare_dram_parameter("wo", [nl, P, CH, CH, P], F16, isOutput=False)
    w1_d = nc.declare_dram_parameter("w1", [nl, P, CH, FCH, P], F16, isOutput=False)
    w2_d = nc.declare_dram_parameter("w2", [nl, CH, P, FCH, P], F16, isOutput=False)
    hw_d = nc.declare_dram_parameter("hw", [P, CH, 2 * NB], F16, isOutput=False)
    out_d = nc.declare_dram_parameter("out", [NB, 2 * NB], F32, isOutput=True)

    from contextlib import ExitStack
    with TileContext(nc) as tc:
        with ExitStack() as ctx:
            persist = ctx.enter_context(tc.tile_pool(name="persist", bufs=1))
            qkv_pool = ctx.enter_context(tc.tile_pool(name="qkvp", bufs=2))
            w768_pool = ctx.enter_context(tc.tile_pool(name="w768", bufs=1))
            w1_pool = ctx.enter_context(tc.tile_pool(name="w1s", bufs=2))
            w2_pool = ctx.enter_context(tc.tile_pool(name="w2s", bufs=2))
            g_pool = ctx.enter_context(tc.tile_pool(name="gp", bufs=1))
            lnp = ctx.enter_context(tc.tile_pool(name="lnp", bufs=5))
            at_pool = ctx.enter_context(tc.tile_pool(name="attn", bufs=4))
            rec_pool = ctx.enter_context(tc.tile_pool(name="recp", bufs=2))
            small_pool = ctx.enter_context(tc.tile_pool(name="small", bufs=2))
            ps_mm = ctx.enter_context(tc.tile_pool(name="psmm", bufs=3, space="PSUM"))
            ps_attn = ctx.enter_context(tc.tile_pool(name="psattn", bufs=5, space="PSUM"))

            ones16 = persist.tile([P, P], F16)
            nc.vector.memset(ones16[:], 1.0)
            eps_t = persist.tile([P, 1], F32)
            nc.vector.memset(eps_t[:], EPS)
            xs = persist.tile([P, CH, T], F16)       # LN2 out: gemm in + master
            ctx_sb = persist.tile([P, CH, T], F16)   # attention context

            def ln16(src16, out16):
                """LayerNorm over features of src16 [P,CH,TB] fp16 -> out16
                fp16. Stats replicated on all partitions via ones-matmuls.
                Stats PSUM comes from ps_attn (idle during phase C) so the
                ps_mm slots stay free for Wo/W1/W2 overlap."""
                ps_s = ps_attn.tile([P, TB], F32, tag="pa")
                for c in range(CH):
                    nc.tensor.matmul(ps_s[:], ones16[:], src16[:, c],
                                     start=(c == 0), stop=(c == CH - 1))
                # uncentered sumsq (var = E[x^2] - m^2): sq runs in parallel
                # with the sum matmuls, shortening the serial chain
                sq = lnp.tile([P, CH, TB], F16, tag="ln")
                nc.vector.tensor_tensor(sq[:], src16[:], src16[:], ALU.mult)
                ps_ss = ps_attn.tile([P, TB], F32, tag="pa")
                for c in range(CH):
                    nc.tensor.matmul(ps_ss[:], ones16[:], sq[:, c],
                                     start=(c == 0), stop=(c == CH - 1))
                mneg = small_pool.tile([P, TB], F16, tag="mneg")
                nc.scalar.activation(mneg[:], ps_s[:], AF.Copy, scale=-1.0 / H)
                msq = small_pool.tile([P, TB], F32, tag="msq")
                nc.scalar.activation(msq[:], mneg[:], AF.Square)
                var = small_pool.tile([P, TB], F32, tag="var")
                nc.vector.scalar_tensor_tensor(var[:], ps_ss[:], 1.0 / H,
                                               msq[:], ALU.mult, ALU.subtract)
                nc.scalar.activation(var[:], var[:], AF.Ln,
                                     bias=eps_t[:, 0:1])
                a_t = small_pool.tile([P, TB], F16, tag="a16")
                nc.scalar.activation(a_t[:], var[:], AF.Exp, scale=-0.5)
                t1 = lnp.tile([P, CH, TB], F16, tag="ln")
                for c in range(CH):
                    nc.vector.tensor_tensor(t1[:, c], src16[:, c], mneg[:],
                                            ALU.add)
                for c in range(CH):
                    nc.vector.tensor_tensor(out16[:, c], t1[:, c], a_t[:],
                                            ALU.mult)

            # ---- embedding LN -> xs (h0 pre-cast to fp16 on host) ----
            for tb in range(NTB):
                sl = slice(tb * TB, (tb + 1) * TB)
                c16 = lnp.tile([P, CH, TB], F16, tag="ln")
                nc.sync.dma_start(c16[:], h0_d[:, :, sl])
                ln16(c16, xs[:, :, sl])

            for l in range(nl):
                wq_t = w768_pool.tile([P, CH, CH, P], F16, tag="wq")
                nc.sync.dma_start(wq_t[:], wq_d[l])
                wk_t = w768_pool.tile([P, CH, CH, P], F16, tag="wk")
                nc.sync.dma_start(wk_t[:], wk_d[l])
                wv_t = w768_pool.tile([P, CH, H], F16, tag="wv")
                nc.sync.dma_start(wv_t[:], wv_d[l])
                wo_t = w768_pool.tile([P, CH, CH, P], F16, tag="wo")
                nc.sync.dma_start(wo_t[:], wo_d[l])

                def do_pair(pr, wq_t=wq_t, wk_t=wk_t, wv_t=wv_t):
                    psl = slice(pr * PAIR, (pr + 1) * PAIR)
                    qt_b = qkv_pool.tile([P, CH, PAIR], F16, tag="qtb")
                    kt_b = qkv_pool.tile([P, CH, PAIR], F16, tag="ktb")
                    v_b = qkv_pool.tile([P, 2, 2, NH, HD], F16, tag="vb")
                    for w_t, dst in ((wq_t, qt_b), (wk_t, kt_b)):
                        for o in range(CH):
                            ps = ps_mm.tile([P, TB], F32, tag="pm")
                            for k in range(CH):
                                nc.tensor.matmul(ps[:], w_t[:, k, o],
                                                 xs[:, k, psl],
                                                 start=(k == 0),
                                                 stop=(k == CH - 1))
                            nc.vector.tensor_copy(dst[:, o], ps[:])
                    for ci in range(PAIR // P):
                        csl = slice(pr * PAIR + ci * P,
                                    pr * PAIR + (ci + 1) * P)
                        bi, kt_i = ci // 2, ci % 2
                        for dh in range(2):
                            ps = ps_mm.tile([P, TB], F32, tag="pm")
                            for k in range(CH):
                                nc.tensor.matmul(
                                    ps[:, : H // 2],
                                    xs[:, k, csl],
                                    wv_t[:, k, dh * (H // 2):(dh + 1) * (H // 2)],
                                    start=(k == 0), stop=(k == CH - 1))
                            nc.vector.tensor_copy(
                                v_b[:, bi, kt_i, dh * 6:(dh + 1) * 6],
                                ps[:, : H // 2].rearrange(
                                    "p (h d) -> p h d", d=HD))

                    # attention over 12 head-pairs (bi, 2j | 2j+1), pipelined
                    hps = [(bi, j) for bi in range(2) for j in range(NH // 2)]

                    def issue_scores(hp):
                        bi, j = hp
                        qsl = slice(bi * S, (bi + 1) * S)
                        ats = []
                        scs = []
                        for hx in range(2):          # po = 0 / 64
                            sc_t = ps_attn.tile([P, 2, S], F32, tag="pa")
                            scs.append(sc_t)
                        for kt_i in range(2):
                            ksl = slice(bi * S + kt_i * P,
                                        bi * S + (kt_i + 1) * P)
                            for hx in range(2):
                                po = hx * HD
                                nc.tensor.matmul(
                                    scs[hx][:, kt_i],
                                    kt_b[po:po + HD, j, ksl],
                                    qt_b[po:po + HD, j, qsl],
                                    start=True, stop=True,
                                    tile_position=(po, 0))
                        for hx in range(2):
                            at = at_pool.tile([P, 2, S], F16, tag="at")
                            nc.scalar.activation(at[:], scs[hx][:], AF.Exp,
                                                 scale=1.0 / np.sqrt(HD))
                            ats.append(at)
                        return ats

                    def issue_rest(hp, ats):
                        bi, j = hp
                        gsl = slice(pr * PAIR + bi * S,
                                    pr * PAIR + (bi + 1) * S)
                        # head A's denominators land on partitions 0:64,
                        # head B's on 64:128 (ones column-slice stationary +
                        # col tile_position) -> one Ln/Exp + one ctx
                        # normalize per head-PAIR
                        ps_sum = ps_attn.tile([P, S], F32, tag="pa")
                        for kt_i in range(2):
                            for hx in range(2):
                                po = hx * HD
                                nc.tensor.matmul(ps_sum[po:po + HD],
                                                 ones16[:, 0:HD],
                                                 ats[hx][:, kt_i],
                                                 start=(kt_i == 0),
                                                 stop=(kt_i == 1),
                                                 tile_position=(0, po))
                        lns = rec_pool.tile([P, S], F32, tag="lns")
                        nc.scalar.activation(lns[:], ps_sum[:], AF.Ln)
                        rec = rec_pool.tile([P, S], F32, tag="rec")
                        nc.scalar.activation(rec[:], lns[:], AF.Exp,
                                             scale=-1.0)
                        ps_ctx = ps_attn.tile([P, S], F32, tag="pa")
                        for kt_i in range(2):
                            for hx in range(2):
                                po = hx * HD
                                nc.tensor.matmul(
                                    ps_ctx[po:po + HD],
                                    v_b[:, bi, kt_i, 2 * j + hx],
                                    ats[hx][:, kt_i],
                                    start=(kt_i == 0), stop=(kt_i == 1),
                                    tile_position=(0, po))
                        nc.vector.tensor_tensor(ctx_sb[:, j, gsl],
                                                ps_ctx[:], rec[:], ALU.mult)

                    pend = {}
                    for i in range(len(hps) + 1):
                        if i < len(hps):
                            pend[i] = issue_scores(hps[i])
                        if i >= 1:
                            issue_rest(hps[i - 1], pend.pop(i - 1))

                def do_cd(tb, l=l, wo_t=wo_t):
                    sl = slice(tb * TB, (tb + 1) * TB)
                    hpre1 = lnp.tile([P, CH, TB], F16, tag="ln")
                    for o in range(CH):
                        ps = ps_mm.tile([P, TB], F32, tag="pm")
                        for k in range(CH):
                            nc.tensor.matmul(ps[:], wo_t[:, k, o],
                                             ctx_sb[:, k, sl],
                                             start=(k == 0), stop=(k == CH - 1))
                        nc.vector.scalar_tensor_tensor(
                            hpre1[:, o], ps[:], 1.0, xs[:, o, sl],
                            ALU.mult, ALU.add)
                    xs1 = lnp.tile([P, CH, TB], F16, tag="ln")
                    ln16(hpre1, xs1[:])
                    g = g_pool.tile([P, FCH, TB], F16, tag="g")
                    for fog in range(FCH // 2):
                        w1_t = w1_pool.tile([P, CH, 2, P], F16, tag="w1")
                        nc.sync.dma_start(
                            w1_t[:], w1_d[l, :, :, fog * 2:(fog + 1) * 2, :])
                        for fi in range(2):
                            fo = fog * 2 + fi
                            ps = ps_mm.tile([P, TB], F32, tag="pm")
                            for k in range(CH):
                                nc.tensor.matmul(ps[:], w1_t[:, k, fi],
                                                 xs1[:, k],
                                                 start=(k == 0),
                                                 stop=(k == CH - 1))
                            nc.scalar.activation(g[:, fo], ps[:], AF.Gelu)
                    hpre2 = lnp.tile([P, CH, TB], F16, tag="ln")
                    HF = FCH // 2
                    for o in range(CH):
                        ps = ps_mm.tile([P, TB], F32, tag="pm")
                        for kh in range(2):
                            w2_t = w2_pool.tile([P, HF, P], F16, tag="w2")
                            nc.sync.dma_start(
                                w2_t[:], w2_d[l, o, :, kh * HF:(kh + 1) * HF])
                            for ki in range(HF):
                                k = kh * HF + ki
                                nc.tensor.matmul(ps[:], w2_t[:, ki], g[:, k],
                                                 start=(k == 0),
                                                 stop=(k == FCH - 1))
                        nc.vector.scalar_tensor_tensor(
                            hpre2[:, o], ps[:], 1.0, xs1[:, o],
                            ALU.mult, ALU.add)
                    ln16(hpre2, xs[:, :, sl])

                for pr in range(T // PAIR):
                    do_pair(pr)
                for tb in range(NTB):
                    do_cd(tb)

            # ---- head ----
            hw_sb = persist.tile([P, CH, 2 * NB], F16)
            nc.sync.dma_start(hw_sb[:], hw_d[:])
            cls = persist.tile([P, CH, NB], F16)
            for c in range(CH):
                nc.vector.tensor_copy(cls[:, c], xs[:, c, 0:T:S])
            ps = ps_attn.tile([P, 2 * NB], F32, tag="pa")
            for c in range(CH):
                nc.tensor.matmul(ps[0:NB], cls[:, c], hw_sb[:, c],
                                 start=(c == 0), stop=(c == CH - 1))
            res = persist.tile([NB, 2 * NB], F32)
            nc.scalar.activation(res[:], ps[0:NB], AF.Copy)
            nc.sync.dma_start(out_d[:], res[:])

    _split_sync_waits(nc, max_waits=1)
    return nc


def _prep_weights(inputs, nl):
    wq = np.stack([_tile_kxo(_f16(inputs["Wq"][i]), H, H) for i in range(nl)])
    wk = np.stack([_tile_kxo(_f16(inputs["Wk"][i]), H, H) for i in range(nl)])
    wv = np.stack([_rows_k(_f16(inputs["Wv"][i]), H, H) for i in range(nl)])
    wo = np.stack([_tile_kxo(_f16(inputs["Wo"][i]), H, H) for i in range(nl)])
    w1 = np.stack([_tile_kxo(_f16(inputs["W1"][i]), H, FF) for i in range(nl)])
    w2 = np.stack([_w2_grouped(_f16(inputs["W2"][i])) for i in range(nl)])
    return wq, wk, wv, wo, w1, w2


def kernel(**inputs):
    nl = _NLAYERS
    for name in ("bq", "bk", "bv", "bo", "b1", "b2", "emb_ln_b", "head_b",
                 "ln1_b", "ln2_b"):
        assert not np.any(np.asarray(inputs[name])), f"{name} nonzero: unsupported"
    for name in ("emb_ln_s", "ln1_s", "ln2_s"):
        assert np.all(np.asarray(inputs[name]) == 1.0), f"{name}!=1: unsupported"
    assert np.all(np.asarray(inputs["attention_mask"]) == 1), "mask unsupported"

    ids = np.asarray(inputs["input_ids"])
    tt = np.asarray(inputs["token_type_ids"])
    we = np.asarray(inputs["word_emb"], np.float32)
    pe = np.asarray(inputs["pos_emb"], np.float32)
    te = np.asarray(inputs["type_emb"], np.float32)
    annot = np.asarray(inputs["annotator_idx"])
    hW = np.asarray(inputs["head_W"], np.float32)

    emb = we[ids] + pe[:S][None] + te[tt]          # [B, S, H] f32
    wq, wk, wv, wo, w1, w2 = _prep_weights(inputs, nl)

    in_maps = []
    for c in range(NCORES):
        e = emb[c * NB:(c + 1) * NB].reshape(T, CH, P).transpose(2, 1, 0)
        hw_g = _f16(hW[annot[c * NB:(c + 1) * NB]])  # [NB, H, 2]
        hwt = hw_g.transpose(1, 0, 2).reshape(H, 2 * NB) \
            .reshape(CH, P, 2 * NB).transpose(1, 0, 2)
        in_maps.append({
            "h0": np.ascontiguousarray(e).astype(np.float16),
            "wq": wq, "wk": wk, "wv": wv, "wo": wo, "w1": w1, "w2": w2,
            "hw": np.ascontiguousarray(hwt),
        })

    nc = build(nl)

    trace = bool(int(os.environ.get("KERNEL_TRACE", "0")))
    kwargs = {}
    if trace:
        try:
            import profshim
            profshim.install()
            kwargs["tmpdir"] = os.environ.get("KERNEL_TRACE_DIR")
        except Exception:
            trace = False
    res = run_bass_kernel_spmd(nc, in_maps, core_ids=list(range(NCORES)),
                               trace=trace, **kwargs)
    kernel.last_exec_time_ns = res.exec_time_ns

    out = np.zeros((B, NL), np.float32)
    for c in range(NCORES):
        oc = res.results[c]["out"]                 # [NB, 2*NB]
        for b in range(NB):
            out[c * NB + b] = oc[b, 2 * b:2 * b + 2]
    return out
